# revision 25
# baseline (speedup 1.0000x reference)
"""BertScore model kernel for Trainium2 (8 NeuronCores, SPMD data-parallel over B).

Reference computation (see problem): cosine-normalized per-layer token reps,
per-(layer,batch) similarity matrix dots = h1 @ h2^T (256x256, contraction
D=1024), ragged masked max over rows/cols + masked means -> s1,s2, F1
harmonic mean -> (B,NL) features, BatchNorm over batch, linear head -> (B,).

Split of work:
- Host: normalization folded into the inputs (h = r/||r||), layout transpose
  to (NL,B,D,L) so the contraction dim D lands on SBUF partitions, additive
  ragged mask rows, and the tiny (B,4) BatchNorm + head epilogue (the
  cross-device batch-stats reduction happens here at gather time).
- Device (per core, 8 batches): 32x [DMA 2 blocks -> 16 accumulating
  matmuls + K=1 mask-row matmul (adds m2[j] to every row) -> DVE max-reduce
  for the row direction -> PE transpose of the 256x256 sim matrix + K=1
  mask-row matmul (adds m1[i]) -> DVE max-reduce for the column direction],
  accumulating 128-wide max vectors into two (128,64) buffers, DMA'd out once.

Masks are applied additively (0 valid / -1e30 invalid). The m2 row added to
the sim matrix also leaks into the transposed path, but it only offsets
whole columns j: valid j columns get +0 (exact) and invalid j columns are
dropped in the host epilogue.

The matmul dtype is selectable: float16 (default; halves DMA traffic, which
is the roofline — end-to-end rel err 6.4e-5) or float32r (full fp32 storage,
fast PE mode, rel err 2.8e-5, ~2x the DMA time).
Input DMA uses a d=8p+q partition mapping so every partition reads
4KB-contiguous runs (measured 1.6x faster than the 512B-run t*128+p mapping).
Measured device time: ~295 us/iteration under a serializing device-side
For_i loop (upper bound; the For_i back-edge defeats cross-iteration
pipelining); cost-model estimate 107.9 us against a ~99 us pure-DMA floor.
"""
import os
import numpy as np

NL, B, L1, L2, D = 4, 64, 256, 256, 1024
NCORES = 8
BB = B // NCORES          # batches per core
KT = D // 128             # contraction tiles
NEG = -1.0e30             # additive mask for invalid positions
BN_EPS = 1e-8
LOGIT_SCALE = 1.0

DTYPE = os.environ.get("BSM_DTYPE", "f16")       # f16 | f32r | f32
REPEAT = int(os.environ.get("BSM_REPEAT", "1"))  # body repeats (for timing)
U = int(os.environ.get("BSM_U", "2"))            # batches merged per DMA
SKIP = set(os.environ.get("BSM_SKIP", "").split(","))  # debug: mm,act,red,dt
IOBUFS = int(os.environ.get("BSM_IOBUFS", "3"))
LOOPN = int(os.environ.get("BSM_LOOPN", "0"))  # >0: wrap body in device For_i loop

_CACHE = {}


def _build(dtype_name, repeat, u, iobufs):
    import concourse.bacc as bacc
    import concourse.bass as bass
    import concourse.mybir as mybir
    import concourse.tile as tile
    from concourse.masks import make_identity

    f32 = mybir.dt.float32
    f32r = mybir.dt.float32r
    dt_in = {
        "f32r": f32r,
        "f16": mybir.dt.float16,
        "f32": f32,
    }[dtype_name]

    nc = bacc.Bacc("TRN2", target_bir_lowering=False, debug=False,
                   num_devices=NCORES)

    h1t = nc.dram_tensor("h1t", [NL, BB, D, L1], dt_in, kind="ExternalInput")
    h2t = nc.dram_tensor("h2t", [NL, BB, D, L2], dt_in, kind="ExternalInput")
    # m1 as per-partition columns (p, b, half): m1c[p,b,h] = m1[b, h*128+p]
    m1c = nc.dram_tensor("m1c", [128, BB, 2], f32, kind="ExternalInput")
    m2d = nc.dram_tensor("m2", [BB, L2], f32r, kind="ExternalInput")
    onesd = nc.dram_tensor("ones", [1, 128], f32r, kind="ExternalInput")
    NCOL = NL * BB * 2
    rmd = nc.dram_tensor("rm", [128, NCOL], f32, kind="ExternalOutput")
    cmd = nc.dram_tensor("cm", [128, NCOL], f32, kind="ExternalOutput")

    with tile.TileContext(nc) as tc:
        with tc.tile_pool(name="consts", bufs=1) as consts, \
             tc.tile_pool(name="io", bufs=iobufs) as io, \
             tc.tile_pool(name="dsbp", bufs=3) as dsbp, \
             tc.tile_pool(name="accp", bufs=1) as accp, \
             tc.tile_pool(name="ps", bufs=3, space="PSUM") as ps, \
             tc.tile_pool(name="psT", bufs=2, space="PSUM") as psT:

            ident = consts.tile([128, 128], f32)
            make_identity(nc, ident)
            ones = consts.tile([1, 128], f32r)
            nc.sync.dma_start(out=ones, in_=onesd.ap())

            # m2 mask rows, one partition: (1, BB, L2); m1 as columns (128, BB, 2)
            m2sb = consts.tile([1, BB, L2], f32r)
            m2ap = m2d.ap()
            nc.sync.dma_start(out=m2sb, in_=bass.AP(
                tensor=m2ap.tensor, offset=m2ap.offset,
                ap=[[0, 1], [L2, BB], [1, L2]]))
            m1sb = consts.tile([128, BB, 2], f32)
            nc.sync.dma_start(out=m1sb, in_=m1c.ap())

            RM = accp.tile([128, NCOL], f32)
            CM = accp.tile([128, NCOL], f32)
            if SKIP & {"mm", "act", "red", "dt"}:
                nc.vector.memset(RM, 0.0)
                nc.vector.memset(CM, 0.0)

            h1ap = h1t.ap()
            h2ap = h2t.ap()
            vmax = mybir.AluOpType.max
            X = mybir.AxisListType.X
            IDENT = mybir.ActivationFunctionType.Identity

            import contextlib
            loop_cm = (tc.For_i(0, LOOPN, 1,
                                hint_engines=(mybir.EngineType.PE,))
                       if LOOPN > 0 else contextlib.nullcontext())
            with loop_cm:
              for _rep in range(repeat):
                for l in range(NL):
                    # d = 8p + q: partition p reads 4KB-contiguous (q, i)
                    src1 = h1ap[l].rearrange("b (p q) i -> p b (q i)", p=128)
                    src2 = h2ap[l].rearrange("b (p q) j -> p b (q j)", p=128)
                    for bu in range(BB // u):
                        h1blk = io.tile([128, u, KT * L1], dt_in, tag="h1")
                        nc.sync.dma_start(
                            out=h1blk, in_=src1[:, bu * u:(bu + 1) * u, :])
                        h2blk = io.tile([128, u, KT * L2], dt_in, tag="h2")
                        nc.sync.dma_start(
                            out=h2blk, in_=src2[:, bu * u:(bu + 1) * u, :])
                        h1v = h1blk.rearrange("p u (q i) -> p u q i", q=KT)
                        h2v = h2blk.rearrange("p u (q j) -> p u q j", q=KT)

                        for ul in range(u):
                            if "mm" in SKIP:
                                continue
                            b = bu * u + ul
                            dsbs = []
                            for it in range(2):
                                dps = ps.tile([128, L2], f32, tag=f"dots{it}")
                                for k in range(KT):
                                    nc.tensor.matmul(
                                        out=dps,
                                        lhsT=h1v[:, ul, k,
                                                  it * 128:(it + 1) * 128],
                                        rhs=h2v[:, ul, k, :],
                                        start=(k == 0), stop=False)
                                # += m2[j] on every row (K=1 accumulate)
                                nc.tensor.matmul(out=dps, lhsT=ones,
                                                 rhs=m2sb[:, b, :],
                                                 start=False, stop=True)
                                # copy PSUM->SBUF with per-partition m1[i]
                                # added (ACT): dsb = dps + m1[i]
                                if "act" in SKIP:
                                    continue
                                dsb = dsbp.tile([128, L2], f32, tag=f"dsb{it}")
                                nc.scalar.activation(
                                    out=dsb, in_=dps, func=IDENT,
                                    bias=m1sb[:, b, it:it + 1])
                                dsbs.append(dsb)
                                # row max: m1[i] is constant along j, so the
                                # masked copy gives the same max for valid i
                                if "red" not in SKIP:
                                    col = (l * BB + b) * 2 + it
                                    nc.vector.tensor_reduce(
                                        out=RM[:, col:col + 1], in_=dsb,
                                        axis=X, op=vmax)

                            if "dt" in SKIP:
                                continue
                            dT = psT.tile([128, 2, L1], f32, tag="dT")
                            for jt in range(2):
                                for it in range(2):
                                    nc.tensor.transpose(
                                        out=dT[:, jt, it * 128:(it + 1) * 128],
                                        in_=dsbs[it][:, jt * 128:(jt + 1) * 128],
                                        identity=ident)
                            for jt in range(2):
                                col = (l * BB + b) * 2 + jt
                                nc.vector.tensor_reduce(
                                    out=CM[:, col:col + 1], in_=dT[:, jt, :],
                                    axis=X, op=vmax)

            for l in range(NL):
                c0, c1 = l * BB * 2, (l + 1) * BB * 2
                nc.sync.dma_start(out=rmd.ap()[:, c0:c1], in_=RM[:, c0:c1])
                nc.sync.dma_start(out=cmd.ap()[:, c0:c1], in_=CM[:, c0:c1])

    nc.finalize()
    return nc


def _get_nc():
    key = (DTYPE, REPEAT, U, IOBUFS, LOOPN, tuple(sorted(SKIP)))
    if key not in _CACHE:
        _CACHE[key] = _build(*key[:4])
    return _CACHE[key]


def _host_prep(reps1, reps2, len1, len2):
    """Normalize, transpose to (NL,B,D,L), build masks; returns per-core maps."""
    np_in = np.float16 if DTYPE == "f16" else np.float32

    def prep(r):
        r = np.asarray(r, dtype=np.float32)
        n = np.sqrt(np.einsum('lbid,lbid->lbi', r, r))
        h = r / n[..., None]
        return np.ascontiguousarray(h.transpose(0, 1, 3, 2)).astype(np_in)

    h1t = prep(reps1)   # (NL, B, D, L1)
    h2t = prep(reps2)
    len1 = np.asarray(len1).astype(np.int64)
    len2 = np.asarray(len2).astype(np.int64)
    ar1 = np.arange(L1)[None, :]
    ar2 = np.arange(L2)[None, :]
    m1 = np.where(ar1 < len1[:, None], 0.0, NEG).astype(np.float32)  # (B, L1)
    m2 = np.where(ar2 < len2[:, None], 0.0, NEG).astype(np.float32)
    # (B, L1) -> (B, 2, 128) -> (128, B, 2)
    m1c = np.ascontiguousarray(m1.reshape(B, 2, 128).transpose(2, 0, 1))

    in_maps = []
    for c in range(NCORES):
        sl = slice(c * BB, (c + 1) * BB)
        in_maps.append({
            "h1t": np.ascontiguousarray(h1t[:, sl]),
            "h2t": np.ascontiguousarray(h2t[:, sl]),
            "m1c": np.ascontiguousarray(m1c[:, sl]),
            "m2": np.ascontiguousarray(m2[sl]),
            "ones": np.ones((1, 128), dtype=np.float32),
        })
    return in_maps, len1, len2


def _epilogue(results, len1, len2, w, b):
    """rm/cm (128, NL*BB*2) per core -> s1,s2 -> F1 -> BatchNorm -> head."""
    maxv_rows = np.empty((NL, B, L1), dtype=np.float64)  # max over valid j, per i
    maxv_cols = np.empty((NL, B, L2), dtype=np.float64)  # max over valid i, per j
    for c, res in enumerate(results):
        rm = np.asarray(res["rm"], dtype=np.float64)  # (128, NCOL)
        cm = np.asarray(res["cm"], dtype=np.float64)
        # column t = (l*BB + b)*2 + half ; partition p -> index half*128 + p
        rm_r = rm.T.reshape(NL, BB, 2, 128).reshape(NL, BB, 256)
        cm_r = cm.T.reshape(NL, BB, 2, 128).reshape(NL, BB, 256)
        maxv_rows[:, c * BB:(c + 1) * BB] = rm_r
        maxv_cols[:, c * BB:(c + 1) * BB] = cm_r

    ar1 = np.arange(L1)[None, :]
    ar2 = np.arange(L2)[None, :]
    mask1 = (ar1 < len1[:, None])  # (B, L1)
    mask2 = (ar2 < len2[:, None])
    n1 = len1.astype(np.float64)
    n2 = len2.astype(np.float64)

    # s2: mean over valid i of (max over valid j); s1: mean over valid j of
    # (max over valid i)
    s2 = np.where(mask1[None], maxv_rows, 0.0).sum(axis=2) / n1[None]  # (NL, B)
    s1 = np.where(mask2[None], maxv_cols, 0.0).sum(axis=2) / n2[None]
    feat = (2.0 * s1 * s2 / (s1 + s2)).T                    # (B, NL)
    mean = feat.mean(axis=0, keepdims=True)
    var = ((feat - mean) ** 2).mean(axis=0, keepdims=True)
    feat = (feat - mean) / np.sqrt(var + BN_EPS)
    w = np.asarray(w, dtype=np.float64)
    bb = np.asarray(b, dtype=np.float64)
    out = LOGIT_SCALE * (feat @ w.T + bb)[:, 0]
    return out.astype(np.float32)


LAST_RUN = {}


def kernel(reps1, reps2, len1, len2, w, b):
    from concourse.bass_utils import run_bass_kernel_spmd

    nc = _get_nc()
    in_maps, l1, l2 = _host_prep(reps1, reps2, len1, len2)
    res = run_bass_kernel_spmd(nc, in_maps, list(range(NCORES)))
    LAST_RUN["results"] = res
    LAST_RUN["in_maps"] = in_maps
    return _epilogue(res.results, l1, l2, w, b)

